# revision 44
# baseline (speedup 1.0000x reference)
"""NT-Xent loss kernel for Trainium2 (8 NeuronCores, Bass/Tile).

Wedge decomposition with host-side normalization and fp8 matmuls.

Host: z = concat(z1, z2), zn = z/||z||, q = fp8e4(16*zn); each core c
receives qT rolled by -1024*c, pre-transposed as [128, 2, 5120]
(partition = d mod 128, dim1 = d half, dim2 = column).  Positive-pair
dots are computed on host in fp64.

Device (per core, rows [0,1024) local = 8 row tiles of 128):
  sim' = 256*cos via fp8e4 DoubleRow matmuls (K=256 in one pass).
  Columns [0,4096) = full block, [4096,4096+128(i+1)) = triangular
  band (keeps distance < 4096; boundary subtile mtri-masked).
  exp handled by two engines in parallel:
   - DVE path (cols [0,1024) of c0 + [0,512) of c1): one
     scalar_tensor_tensor per window converts PSUM logits straight to
     bf16 BITS (Schraudolph: i16 = sat(round(psum*s1 + master))),
     masking the self-sim diagonal via the `master` window; one
     reduce_sum over both windows yields row sums.
   - ACT path (the rest): Exp activation with accum_out row sums,
     fp8e5 E output.
  Column sums credit mirror rows via ones-matmul chains (fp8e5
  DoubleRow row-tile pairs / plain bf16), split into half-chains
  (tiles 0-3 / 4-7) that the host adds together, so the first halves
  overlap the main loop and the tail stays short.
Host un-rotates, adds exp(10*pos) in fp64, takes log + mean.
"""

import sys

if "/opt/trn_rl_repo" not in sys.path:
    sys.path.insert(0, "/opt/trn_rl_repo")

import numpy as np
import ml_dtypes

import concourse.bacc as bacc
import concourse.mybir as mybir
import concourse.tile as tile

P = 128
D = 256
M = 8192
N2 = 4096
NCORES = 8
IT = 8               # row tiles per core
WCOL = 5120          # columns of znT each core needs
TEMP_INV = 10.0
QS = 16.0            # host quantization scale: q = fp8(QS * zn)
SC = TEMP_INV / (QS * QS)        # ACT exp scale: exp(SC * sim')
LN2 = float(np.log(2.0))
S1D = SC * 128.0 / LN2           # DVE bf16-schraudolph slope
S2D = 16248.64                   # bias (incl. mean-error calibration)
S1D8 = SC * 4.0 / LN2            # fp8e5-schraudolph slope (c1 window)
S2D8 = 59.7787
MASKV = -1536.0                  # band mask in PSUM units: SC*MASKV = -60
MASKD = -40000.0                 # diag mask added to master (x' < 0 -> 0)
W0 = 1024            # DVE width in c0 (covers all diag subtiles)
W2 = 512             # DVE width in c1
WD = W0 + W2

F32 = mybir.dt.float32
BF16 = mybir.dt.bfloat16
FP8E4 = mybir.dt.float8e4
FP8E5 = mybir.dt.float8e5
I16 = mybir.dt.int16
I8 = mybir.dt.int8
AF = mybir.ActivationFunctionType
ALU = mybir.AluOpType
DR = mybir.MatmulPerfMode.DoubleRow

# colsum pieces: (cc, src, off, t0, nt) -> cs_sb[:, 512*p : 512*(p+1)]
# covers local col chunk 1024 + 512*cc over row tiles [t0, t0+nt);
# host adds the pieces of each chunk.
#   src: 0=E0a (fp8), 1=Ed bf16 (c1 window, off absolute), 2=E1a, 3=Eb
_CHUNKS = [
    (0, 0, 0), (1, 0, 512), (2, 4, 0), (3, 2, 0),
    (4, 2, 512), (5, 2, 1024), (6, 3, 0),
]
PIECES = (
    [(cc, src, off, 0, 4) for cc, src, off in _CHUNKS]     # A: tiles 0-3
    + [(cc, src, off, 4, 4) for cc, src, off in _CHUNKS]   # B: tiles 4-7
    + [(7, 3, 512, 4, 4)]
)
NP_ = len(PIECES)

_nc_cache = None


def _build():
    nc = bacc.Bacc(None, target_bir_lowering=False)
    zt = nc.dram_tensor("zt", [P, 2, WCOL], FP8E4, kind="ExternalInput")
    mtri_in = nc.dram_tensor("mtri", [P, P], F32, kind="ExternalInput")
    maskd_in = nc.dram_tensor("maskd", [P, P], F32, kind="ExternalInput")
    out_acc = nc.dram_tensor("out_acc", [P, IT * 6], F32, kind="ExternalOutput")
    out_cs = nc.dram_tensor("out_cs", [1, NP_ * 512], BF16, kind="ExternalOutput")

    with (
        tile.TileContext(nc) as tc,
        tc.tile_pool(name="sb", bufs=1) as sb,
        tc.tile_pool(name="big", bufs=3, space="PSUM") as bigp,
        tc.tile_pool(name="csp", bufs=1, space="PSUM") as cspp,
    ):
        zts = sb.tile([P, 2, WCOL], FP8E4)
        Ed = sb.tile([P, IT, W0], BF16)        # c0 DVE E (schraudolph bits)
        E1d = sb.tile([P, IT, W2], FP8E5)      # c1 DVE E (fp8e5 bits)
        E0a = sb.tile([P, IT, 2048 - W0], FP8E5)
        E1a = sb.tile([P, IT, 2048 - W2], FP8E5)
        Eb = sb.tile([P, IT, 1024], FP8E5)
        acc = sb.tile([P, IT, 6], F32)
        cs_sb = sb.tile([P, NP_ * 512], BF16)
        master = sb.tile([P, 2048], F32)
        mtri = sb.tile([P, P], F32)
        maskd = sb.tile([P, P], F32)
        ones2 = sb.tile([P, 2, P], FP8E5)      # DoubleRow colsum lhsT
        ones1b = sb.tile([P, P], BF16)         # plain bf16 colsum lhsT

        # memsets first (no deps; master must beat the first STT)
        nc.vector.memset(master, float(S2D))
        nc.vector.memset(acc, 0.0)
        nc.vector.memset(ones2, 1.0)
        nc.vector.memset(ones1b, 1.0)

        # input DMAs on the two fast HWDGE queues, ordered by first use
        # (SWDGE/gpsimd descriptor gen is too slow for the early slabs)
        nc.sync.dma_start(out=zts[:, :, 0:512], in_=zt[:, :, 0:512])
        nc.sync.dma_start(out=zts[:, :, 512:1024], in_=zt[:, :, 512:1024])
        nc.scalar.dma_start(out=zts[:, :, 1024:2048], in_=zt[:, :, 1024:2048])
        nc.scalar.dma_start(out=zts[:, :, 2048:3072], in_=zt[:, :, 2048:3072])
        nc.sync.dma_start(out=zts[:, :, 3072:4096], in_=zt[:, :, 3072:4096])
        nc.scalar.dma_start(out=zts[:, :, 4096:WCOL], in_=zt[:, :, 4096:WCOL])
        nc.sync.dma_start(out=maskd, in_=maskd_in[:, :])
        nc.sync.dma_start(out=mtri, in_=mtri_in[:, :])
        nc.gpsimd.memset(Eb, 0.0)   # band tail stays zero for full-width cs

        # master: S2D everywhere; diag-mask block at cols [1024, 1152)
        # (row tile i's STT window [1024-128i, ...) puts the block exactly
        # over its self-sim diagonal subtile)
        nc.vector.tensor_add(master[:, 1024:1152], master[:, 1024:1152], maskd)

        cps1 = cspp.tile([P, 512], F32, name="cps1")
        cps2 = cspp.tile([P, 512], F32, name="cps2")
        cps = [cps1, cps2]

        def lhsT(i):
            return zts[:, :, i * P : (i + 1) * P]

        def emit_main(i):
            # ---- c0: cols [0, 2048) ----
            a0 = bigp.tile([P, W0], F32, tag="ps", name=f"a0_{i}")
            a1 = bigp.tile([P, 2048 - W0], F32, tag="ps", name=f"a1_{i}")
            for n in range(0, W0, 512):
                nc.tensor.matmul(
                    a0[:, n : n + 512], lhsT=lhsT(i),
                    rhs=zts[:, :, n : n + 512],
                    start=True, stop=True, perf_mode=DR,
                )
            for n in range(W0, 2048, 512):
                nc.tensor.matmul(
                    a1[:, n - W0 : n - W0 + 512], lhsT=lhsT(i),
                    rhs=zts[:, :, n : n + 512],
                    start=True, stop=True, perf_mode=DR,
                )
            # DVE: bf16 schraudolph, diag mask fused via master window
            nc.vector.scalar_tensor_tensor(
                out=Ed[:, i, 0:W0].bitcast(I16), in0=a0[:, :],
                scalar=float(S1D),
                in1=master[:, 1024 - P * i : 1024 - P * i + W0],
                op0=ALU.mult, op1=ALU.add,
            )
            nc.scalar.activation(
                out=E0a[:, i, :], in_=a1[:, :], func=AF.Exp, scale=float(SC),
                accum_out=acc[:, i, 1:2],
            )

            # ---- c1: cols [2048, 4096) ----
            b0 = bigp.tile([P, 1024], F32, tag="ps", name=f"b0_{i}")
            b1 = bigp.tile([P, 1024], F32, tag="ps", name=f"b1_{i}")
            for n in range(0, 1024, 512):
                nc.tensor.matmul(
                    b0[:, n : n + 512], lhsT=lhsT(i),
                    rhs=zts[:, :, 2048 + n : 2048 + n + 512],
                    start=True, stop=True, perf_mode=DR,
                )
            for n in range(0, 1024, 512):
                nc.tensor.matmul(
                    b1[:, n : n + 512], lhsT=lhsT(i),
                    rhs=zts[:, :, 3072 + n : 3072 + n + 512],
                    start=True, stop=True, perf_mode=DR,
                )
            nc.vector.tensor_scalar(
                out=E1d[:, i, :].bitcast(I8), in0=b0[:, 0:W2],
                scalar1=float(S1D8), scalar2=float(S2D8),
                op0=ALU.mult, op1=ALU.add,
            )
            nc.vector.reduce_sum(
                acc[:, i, 0:1], Ed[:, i, :], axis=mybir.AxisListType.X
            )
            nc.vector.reduce_sum(
                acc[:, i, 2:3], E1d[:, i, :], axis=mybir.AxisListType.X
            )
            nc.scalar.activation(
                out=E1a[:, i, 0 : 1024 - W2], in_=b0[:, W2:1024],
                func=AF.Exp, scale=float(SC), accum_out=acc[:, i, 3:4],
            )
            nc.scalar.activation(
                out=E1a[:, i, 1024 - W2 : 2048 - W2], in_=b1[:, :],
                func=AF.Exp, scale=float(SC), accum_out=acc[:, i, 4:5],
            )

            # ---- band: cols [4096, 4096 + 128(i+1)) ----
            wb = P * (i + 1)
            cb = bigp.tile([P, 1024], F32, tag="ps", name=f"cb_{i}")
            full = wb - P
            n = 0
            while n < full:
                w = min(512 - (n % 512), full - n)
                nc.tensor.matmul(
                    cb[:, n : n + w], lhsT=lhsT(i),
                    rhs=zts[:, :, 4096 + n : 4096 + n + w],
                    start=True, stop=True, perf_mode=DR,
                )
                n += w
            nc.tensor.matmul(
                cb[:, full : full + P], lhsT=lhsT(i),
                rhs=zts[:, :, 4096 + full : 4096 + full + P],
                start=True, stop=True, perf_mode=DR,
            )
            nc.vector.tensor_add(
                cb[:, full : full + P], cb[:, full : full + P], mtri
            )
            nc.scalar.activation(
                out=Eb[:, i, 0:wb], in_=cb[:, 0:wb], func=AF.Exp,
                scale=float(SC), accum_out=acc[:, i, 5:6],
            )

        def emit_piece(p, bank, engine):
            cc, src, off, t0, nt = PIECES[p]
            dst = cps[bank]
            if src == 1:
                # bf16 plain chain over the piece's row tiles
                for k in range(nt):
                    nc.tensor.matmul(
                        dst[:, :], lhsT=ones1b,
                        rhs=Ed[:, t0 + k, off : off + 512],
                        start=(k == 0), stop=(k == nt - 1),
                    )
            else:
                S = {0: E0a, 2: E1a, 3: Eb, 4: E1d}[src]
                for k in range(nt // 2):
                    nc.tensor.matmul(
                        dst[:, :], lhsT=ones2,
                        rhs=S[:, t0 + 2 * k : t0 + 2 * k + 2, off : off + 512],
                        start=(k == 0), stop=(k == nt // 2 - 1), perf_mode=DR,
                    )
            seg = cs_sb[:, 512 * p : 512 * (p + 1)]
            if engine == "v":
                nc.vector.tensor_copy(out=seg, in_=dst[:, :])
            else:
                nc.scalar.copy(seg, dst[:, :])

        # ---- main schedule: A pieces (tiles 0-3) ride iters 4-7,
        # emitted BEFORE the iter so their copies queue ahead of the
        # iter's exp work and free the cps banks sooner ----
        SCHED = {4: [0, 1], 5: [2, 3], 6: [4, 5], 7: [6]}
        for i in range(IT):
            for j, p in enumerate(SCHED.get(i, [])):
                emit_piece(p, j % 2, "v")
            emit_main(i)
        # tail: B pieces (tiles 4-7), copies split across both exp engines
        for j, p in enumerate([14] + list(range(7, 14))):
            emit_piece(p, j % 2, "v" if j % 2 else "s")

        nc.scalar.dma_start(
            out=out_acc[:, :], in_=acc.rearrange("p i s -> p (i s)")
        )
        nc.sync.dma_start(out=out_cs[0:1, :], in_=cs_sb[0:1, :])

    nc.finalize()
    return nc


def _get_nc():
    global _nc_cache
    if _nc_cache is None:
        _nc_cache = _build()
    return _nc_cache


def _prep_inputs(z: np.ndarray):
    """z: [M, D] float32 (unnormalized). Returns per-core input maps and
    host-side positive-pair cosines."""
    nrm = np.sqrt((z.astype(np.float64) ** 2).sum(axis=1))
    zn = z / np.maximum(nrm, 1e-8).astype(np.float32)[:, None]
    zn64 = zn.astype(np.float64)
    pos = (zn64 * np.roll(zn64, -N2, axis=0)).sum(axis=1)
    q = (QS * zn).astype(ml_dtypes.float8_e4m3)
    # [128, 2, 8192]: [p, h, col] = q[col, 128h + p]
    big = np.ascontiguousarray(q.T.reshape(2, P, M).transpose(1, 0, 2))
    mtri = np.where(
        np.arange(P)[None, :] >= np.arange(P)[:, None], MASKV, 0.0
    ).astype(np.float32)
    maskd = (MASKD * np.eye(P)).astype(np.float32)
    in_maps = []
    for c in range(NCORES):
        zr = np.roll(big, -1024 * c, axis=2)[:, :, :WCOL]
        in_maps.append(
            {"zt": np.ascontiguousarray(zr), "mtri": mtri, "maskd": maskd}
        )
    return in_maps, pos


def _run_cores(z: np.ndarray, trace: bool = False):
    from concourse.bass_utils import run_bass_kernel_spmd

    nc = _get_nc()
    in_maps, _ = _prep_inputs(np.asarray(z, np.float32))
    return run_bass_kernel_spmd(
        nc, in_maps, core_ids=list(range(NCORES)), trace=trace
    )


def _combine(results, pos):
    total = np.zeros(M, np.float64)
    idx = np.arange(512)
    for c, r in enumerate(results):
        accv = np.asarray(r["out_acc"]).astype(np.float64).reshape(P, IT, 6)
        cs = np.asarray(r["out_cs"]).astype(np.float64)[0]
        base = 1024 * c
        rows = accv.sum(axis=2)  # [P, IT]
        for i in range(IT):
            g = (base + i * P + np.arange(P)) % M
            total[g] += rows[:, i]
        for p, (cc, src, off, t0, nt) in enumerate(PIECES):
            j = 1024 + 512 * cc + idx
            total[(base + j) % M] += cs[512 * p : 512 * (p + 1)]

    total += np.exp(TEMP_INV * pos)
    lse = np.log(total)
    return np.float32((lse - TEMP_INV * pos).mean())


def kernel(z1: np.ndarray, z2: np.ndarray) -> np.ndarray:
    from concourse.bass_utils import run_bass_kernel_spmd

    z = np.concatenate(
        [np.asarray(z1, np.float32), np.asarray(z2, np.float32)], axis=0
    )
    nc = _get_nc()
    in_maps, pos = _prep_inputs(z)
    res = run_bass_kernel_spmd(nc, in_maps, core_ids=list(range(NCORES)))
    return _combine(res.results, pos)


# revision 45
# speedup vs baseline: 1.0416x; 1.0416x over previous
"""NT-Xent loss kernel for Trainium2 (8 NeuronCores, Bass/Tile).

Wedge decomposition with host-side normalization and fp8 matmuls.

Host: z = concat(z1, z2), zn = z/||z||, q = fp8e4(16*zn); each core c
receives qT rolled by -1024*c, pre-transposed as [128, 2, 5120]
(partition = d mod 128, dim1 = d half, dim2 = column).  Positive-pair
dots are computed on host in fp64.

Device (per core, rows [0,1024) local = 8 row tiles of 128):
  sim' = 256*cos via fp8e4 DoubleRow matmuls (K=256 in one pass).
  Columns [0,4096) = full block, [4096,4096+128(i+1)) = triangular
  band (keeps distance < 4096; boundary subtile mtri-masked).
  exp handled by two engines in parallel:
   - DVE path (cols [0,1024) of c0 + [0,512) of c1): one
     scalar_tensor_tensor per window converts PSUM logits straight to
     bf16 BITS (Schraudolph: i16 = sat(round(psum*s1 + master))),
     masking the self-sim diagonal via the `master` window; one
     reduce_sum over both windows yields row sums.
   - ACT path (the rest): Exp activation with accum_out row sums,
     fp8e5 E output.
  Column sums credit mirror rows via ones-matmul chains (fp8e5
  DoubleRow row-tile pairs / plain bf16), split into half-chains
  (tiles 0-3 / 4-7) that the host adds together, so the first halves
  overlap the main loop and the tail stays short.
Host un-rotates, adds exp(10*pos) in fp64, takes log + mean.
"""

import sys

if "/opt/trn_rl_repo" not in sys.path:
    sys.path.insert(0, "/opt/trn_rl_repo")

import numpy as np
import ml_dtypes

import concourse.bacc as bacc
import concourse.mybir as mybir
import concourse.tile as tile

P = 128
D = 256
M = 8192
N2 = 4096
NCORES = 8
IT = 8               # row tiles per core
WCOL = 5120          # columns of znT each core needs
TEMP_INV = 10.0
QS = 16.0            # host quantization scale: q = fp8(QS * zn)
SC = TEMP_INV / (QS * QS)        # ACT exp scale: exp(SC * sim')
LN2 = float(np.log(2.0))
S1D = SC * 128.0 / LN2           # DVE bf16-schraudolph slope
S2D = 16248.64                   # bias (incl. mean-error calibration)
S1D8 = SC * 4.0 / LN2            # fp8e5-schraudolph slope (c1 window)
S2D8 = 59.7787
MASKV = -1536.0                  # band mask in PSUM units: SC*MASKV = -60
MASKD = -40000.0                 # diag mask added to master (x' < 0 -> 0)
W0 = 1024            # DVE width in c0 (covers all diag subtiles)
W2 = 512             # DVE width in c1
WD = W0 + W2

F32 = mybir.dt.float32
BF16 = mybir.dt.bfloat16
FP8E4 = mybir.dt.float8e4
FP8E5 = mybir.dt.float8e5
I16 = mybir.dt.int16
I8 = mybir.dt.int8
AF = mybir.ActivationFunctionType
ALU = mybir.AluOpType
DR = mybir.MatmulPerfMode.DoubleRow

# colsum pieces: (cc, src, off, t0, nt) -> cs_sb[:, 512*p : 512*(p+1)]
# covers local col chunk 1024 + 512*cc over row tiles [t0, t0+nt);
# host adds the pieces of each chunk.
#   src: 0=E0a (fp8), 1=Ed bf16 (c1 window, off absolute), 2=E1a, 3=Eb
_CHUNKS = [
    (0, 0, 0), (1, 0, 512), (2, 4, 0), (3, 2, 0),
    (4, 2, 512), (5, 2, 1024), (6, 3, 0),
]
PIECES = (
    [(cc, src, off, 0, 4) for cc, src, off in _CHUNKS]     # A: tiles 0-3
    + [(cc, src, off, 4, 4) for cc, src, off in _CHUNKS]   # B: tiles 4-7
    + [(7, 3, 512, 4, 4)]
)
NP_ = len(PIECES)

_nc_cache = None


def _build():
    nc = bacc.Bacc(None, target_bir_lowering=False)
    zt = nc.dram_tensor("zt", [P, 2, WCOL], FP8E4, kind="ExternalInput")
    mtri_in = nc.dram_tensor("mtri", [P, P], F32, kind="ExternalInput")
    maskd_in = nc.dram_tensor("maskd", [P, P], F32, kind="ExternalInput")
    out_acc = nc.dram_tensor("out_acc", [P, IT * 6], F32, kind="ExternalOutput")
    out_cs = nc.dram_tensor("out_cs", [1, NP_ * 512], BF16, kind="ExternalOutput")

    with (
        tile.TileContext(nc) as tc,
        tc.tile_pool(name="sb", bufs=1) as sb,
        tc.tile_pool(name="big", bufs=3, space="PSUM") as bigp,
        tc.tile_pool(name="csp", bufs=1, space="PSUM") as cspp,
    ):
        zts = sb.tile([P, 2, WCOL], FP8E4)
        Ed = sb.tile([P, IT, W0], BF16)        # c0 DVE E (schraudolph bits)
        E1d = sb.tile([P, IT, W2], FP8E5)      # c1 DVE E (fp8e5 bits)
        E0a = sb.tile([P, IT, 2048 - W0], FP8E5)
        E1a = sb.tile([P, IT, 2048 - W2], FP8E5)
        Eb = sb.tile([P, IT, 1024], FP8E5)
        acc = sb.tile([P, IT, 6], F32)
        cs_sb = sb.tile([P, NP_ * 512], BF16)
        master = sb.tile([P, 2048], F32)
        mtri = sb.tile([P, P], F32)
        maskd = sb.tile([P, P], F32)
        ones2 = sb.tile([P, 2, P], FP8E5)      # DoubleRow colsum lhsT
        ones1b = sb.tile([P, P], BF16)         # plain bf16 colsum lhsT

        # memsets first (no deps; master must beat the first STT)
        nc.vector.memset(master, float(S2D))
        nc.vector.memset(acc, 0.0)
        nc.vector.memset(ones2, 1.0)
        nc.vector.memset(ones1b, 1.0)

        # input DMAs on the two fast HWDGE queues, ordered by first use
        # (SWDGE/gpsimd descriptor gen is too slow for the early slabs)
        nc.sync.dma_start(out=zts[:, :, 0:512], in_=zt[:, :, 0:512])
        nc.sync.dma_start(out=zts[:, :, 512:1024], in_=zt[:, :, 512:1024])
        nc.scalar.dma_start(out=zts[:, :, 1024:2048], in_=zt[:, :, 1024:2048])
        nc.scalar.dma_start(out=zts[:, :, 2048:3072], in_=zt[:, :, 2048:3072])
        nc.sync.dma_start(out=zts[:, :, 3072:4096], in_=zt[:, :, 3072:4096])
        nc.scalar.dma_start(out=zts[:, :, 4096:WCOL], in_=zt[:, :, 4096:WCOL])
        nc.sync.dma_start(out=maskd, in_=maskd_in[:, :])
        nc.sync.dma_start(out=mtri, in_=mtri_in[:, :])
        nc.gpsimd.memset(Eb, 0.0)   # band tail stays zero for full-width cs

        # master: S2D everywhere; diag-mask block at cols [1024, 1152)
        # (row tile i's STT window [1024-128i, ...) puts the block exactly
        # over its self-sim diagonal subtile)
        nc.vector.tensor_add(master[:, 1024:1152], master[:, 1024:1152], maskd)

        cps1 = cspp.tile([P, 512], F32, name="cps1")
        cps2 = cspp.tile([P, 512], F32, name="cps2")
        cps = [cps1, cps2]

        def lhsT(i):
            return zts[:, :, i * P : (i + 1) * P]

        def emit_main(i):
            # ---- c0: cols [0, 2048) ----
            a0 = bigp.tile([P, W0], F32, tag="ps", name=f"a0_{i}")
            a1 = bigp.tile([P, 2048 - W0], F32, tag="ps", name=f"a1_{i}")
            for n in range(0, W0, 512):
                nc.tensor.matmul(
                    a0[:, n : n + 512], lhsT=lhsT(i),
                    rhs=zts[:, :, n : n + 512],
                    start=True, stop=True, perf_mode=DR,
                )
            for n in range(W0, 2048, 512):
                nc.tensor.matmul(
                    a1[:, n - W0 : n - W0 + 512], lhsT=lhsT(i),
                    rhs=zts[:, :, n : n + 512],
                    start=True, stop=True, perf_mode=DR,
                )
            # DVE: bf16 schraudolph, diag mask fused via master window
            nc.vector.scalar_tensor_tensor(
                out=Ed[:, i, 0:W0].bitcast(I16), in0=a0[:, :],
                scalar=float(S1D),
                in1=master[:, 1024 - P * i : 1024 - P * i + W0],
                op0=ALU.mult, op1=ALU.add,
            )
            nc.scalar.activation(
                out=E0a[:, i, :], in_=a1[:, :], func=AF.Exp, scale=float(SC),
                accum_out=acc[:, i, 1:2],
            )

            # ---- c1: cols [2048, 4096) ----
            b0 = bigp.tile([P, 1024], F32, tag="ps", name=f"b0_{i}")
            b1 = bigp.tile([P, 1024], F32, tag="ps", name=f"b1_{i}")
            for n in range(0, 1024, 512):
                nc.tensor.matmul(
                    b0[:, n : n + 512], lhsT=lhsT(i),
                    rhs=zts[:, :, 2048 + n : 2048 + n + 512],
                    start=True, stop=True, perf_mode=DR,
                )
            for n in range(0, 1024, 512):
                nc.tensor.matmul(
                    b1[:, n : n + 512], lhsT=lhsT(i),
                    rhs=zts[:, :, 3072 + n : 3072 + n + 512],
                    start=True, stop=True, perf_mode=DR,
                )
            nc.vector.tensor_scalar(
                out=E1d[:, i, :].bitcast(I8), in0=b0[:, 0:W2],
                scalar1=float(S1D8), scalar2=float(S2D8),
                op0=ALU.mult, op1=ALU.add,
            )
            nc.vector.reduce_sum(
                acc[:, i, 0:1], Ed[:, i, :], axis=mybir.AxisListType.X
            )
            nc.vector.reduce_sum(
                acc[:, i, 2:3], E1d[:, i, :], axis=mybir.AxisListType.X
            )
            nc.scalar.activation(
                out=E1a[:, i, 0 : 1024 - W2], in_=b0[:, W2:1024],
                func=AF.Exp, scale=float(SC), accum_out=acc[:, i, 3:4],
            )
            nc.scalar.activation(
                out=E1a[:, i, 1024 - W2 : 2048 - W2], in_=b1[:, :],
                func=AF.Exp, scale=float(SC), accum_out=acc[:, i, 4:5],
            )

            # ---- band: cols [4096, 4096 + 128(i+1)) ----
            wb = P * (i + 1)
            cb = bigp.tile([P, 1024], F32, tag="ps", name=f"cb_{i}")
            full = wb - P
            n = 0
            while n < full:
                w = min(512 - (n % 512), full - n)
                nc.tensor.matmul(
                    cb[:, n : n + w], lhsT=lhsT(i),
                    rhs=zts[:, :, 4096 + n : 4096 + n + w],
                    start=True, stop=True, perf_mode=DR,
                )
                n += w
            nc.tensor.matmul(
                cb[:, full : full + P], lhsT=lhsT(i),
                rhs=zts[:, :, 4096 + full : 4096 + full + P],
                start=True, stop=True, perf_mode=DR,
            )
            nc.vector.tensor_add(
                cb[:, full : full + P], cb[:, full : full + P], mtri
            )
            nc.scalar.activation(
                out=Eb[:, i, 0:wb], in_=cb[:, 0:wb], func=AF.Exp,
                scale=float(SC), accum_out=acc[:, i, 5:6],
            )

        def emit_piece(p, bank, engine):
            cc, src, off, t0, nt = PIECES[p]
            dst = cps[bank]
            if src == 1:
                # bf16 plain chain over the piece's row tiles
                for k in range(nt):
                    nc.tensor.matmul(
                        dst[:, :], lhsT=ones1b,
                        rhs=Ed[:, t0 + k, off : off + 512],
                        start=(k == 0), stop=(k == nt - 1),
                    )
            else:
                S = {0: E0a, 2: E1a, 3: Eb, 4: E1d}[src]
                for k in range(nt // 2):
                    nc.tensor.matmul(
                        dst[:, :], lhsT=ones2,
                        rhs=S[:, t0 + 2 * k : t0 + 2 * k + 2, off : off + 512],
                        start=(k == 0), stop=(k == nt // 2 - 1), perf_mode=DR,
                    )
            seg = cs_sb[:, 512 * p : 512 * (p + 1)]
            if engine == "v":
                nc.vector.tensor_copy(out=seg, in_=dst[:, :])
            else:
                nc.scalar.copy(seg, dst[:, :])

        # ---- main schedule: A pieces (tiles 0-3) ride iters 4-7 ----
        SCHED = {4: [0, 1], 5: [2, 3], 6: [4, 5], 7: [6]}
        for i in range(IT):
            emit_main(i)
            for j, p in enumerate(SCHED.get(i, [])):
                emit_piece(p, j % 2, "v")
        # tail: B pieces (tiles 4-7), copies split across both exp engines
        for j, p in enumerate([14] + list(range(7, 14))):
            emit_piece(p, j % 2, "v" if j % 2 else "s")

        nc.scalar.dma_start(
            out=out_acc[:, :], in_=acc.rearrange("p i s -> p (i s)")
        )
        nc.sync.dma_start(out=out_cs[0:1, :], in_=cs_sb[0:1, :])

    nc.finalize()
    return nc


def _get_nc():
    global _nc_cache
    if _nc_cache is None:
        _nc_cache = _build()
    return _nc_cache


def _prep_inputs(z: np.ndarray):
    """z: [M, D] float32 (unnormalized). Returns per-core input maps and
    host-side positive-pair cosines."""
    nrm = np.sqrt((z.astype(np.float64) ** 2).sum(axis=1))
    zn = z / np.maximum(nrm, 1e-8).astype(np.float32)[:, None]
    zn64 = zn.astype(np.float64)
    pos = (zn64 * np.roll(zn64, -N2, axis=0)).sum(axis=1)
    q = (QS * zn).astype(ml_dtypes.float8_e4m3)
    # [128, 2, 8192]: [p, h, col] = q[col, 128h + p]
    big = np.ascontiguousarray(q.T.reshape(2, P, M).transpose(1, 0, 2))
    mtri = np.where(
        np.arange(P)[None, :] >= np.arange(P)[:, None], MASKV, 0.0
    ).astype(np.float32)
    maskd = (MASKD * np.eye(P)).astype(np.float32)
    in_maps = []
    for c in range(NCORES):
        zr = np.roll(big, -1024 * c, axis=2)[:, :, :WCOL]
        in_maps.append(
            {"zt": np.ascontiguousarray(zr), "mtri": mtri, "maskd": maskd}
        )
    return in_maps, pos


def _run_cores(z: np.ndarray, trace: bool = False):
    from concourse.bass_utils import run_bass_kernel_spmd

    nc = _get_nc()
    in_maps, _ = _prep_inputs(np.asarray(z, np.float32))
    return run_bass_kernel_spmd(
        nc, in_maps, core_ids=list(range(NCORES)), trace=trace
    )


def _combine(results, pos):
    total = np.zeros(M, np.float64)
    idx = np.arange(512)
    for c, r in enumerate(results):
        accv = np.asarray(r["out_acc"]).astype(np.float64).reshape(P, IT, 6)
        cs = np.asarray(r["out_cs"]).astype(np.float64)[0]
        base = 1024 * c
        rows = accv.sum(axis=2)  # [P, IT]
        for i in range(IT):
            g = (base + i * P + np.arange(P)) % M
            total[g] += rows[:, i]
        for p, (cc, src, off, t0, nt) in enumerate(PIECES):
            j = 1024 + 512 * cc + idx
            total[(base + j) % M] += cs[512 * p : 512 * (p + 1)]

    total += np.exp(TEMP_INV * pos)
    lse = np.log(total)
    return np.float32((lse - TEMP_INV * pos).mean())


def kernel(z1: np.ndarray, z2: np.ndarray) -> np.ndarray:
    from concourse.bass_utils import run_bass_kernel_spmd

    z = np.concatenate(
        [np.asarray(z1, np.float32), np.asarray(z2, np.float32)], axis=0
    )
    nc = _get_nc()
    in_maps, pos = _prep_inputs(z)
    res = run_bass_kernel_spmd(nc, in_maps, core_ids=list(range(NCORES)))
    return _combine(res.results, pos)


# revision 47
# speedup vs baseline: 1.0531x; 1.0110x over previous
"""NT-Xent loss kernel for Trainium2 (8 NeuronCores, Bass/Tile).

Wedge decomposition with host-side normalization and fp8 matmuls.

Host: z = concat(z1, z2), zn = z/||z||, q = fp8e4(16*zn); each core c
receives qT rolled by -1024*c, pre-transposed as [128, 2, 5120]
(partition = d mod 128, dim1 = d half, dim2 = column).  Positive-pair
dots are computed on host in fp64.

Device (per core, rows [0,1024) local = 8 row tiles of 128):
  sim' = 256*cos via fp8e4 DoubleRow matmuls (K=256 in one pass).
  Columns [0,4096) = full block, [4096,4096+128(i+1)) = triangular
  band (keeps distance < 4096; boundary subtile mtri-masked).
  exp handled by two engines in parallel:
   - DVE path (cols [0,1024) of c0 + [0,512) of c1): one
     scalar_tensor_tensor per window converts PSUM logits straight to
     bf16 BITS (Schraudolph: i16 = sat(round(psum*s1 + master))),
     masking the self-sim diagonal via the `master` window; one
     reduce_sum over both windows yields row sums.
   - ACT path (the rest): Exp activation with accum_out row sums,
     fp8e5 E output.
  Column sums credit mirror rows via ones-matmul chains (fp8e5
  DoubleRow row-tile pairs / plain bf16), split into half-chains
  (tiles 0-3 / 4-7) that the host adds together, so the first halves
  overlap the main loop and the tail stays short.
Host un-rotates, adds exp(10*pos) in fp64, takes log + mean.
"""

import sys

if "/opt/trn_rl_repo" not in sys.path:
    sys.path.insert(0, "/opt/trn_rl_repo")

import numpy as np
import ml_dtypes

import concourse.bacc as bacc
import concourse.mybir as mybir
import concourse.tile as tile

P = 128
D = 256
M = 8192
N2 = 4096
NCORES = 8
IT = 8               # row tiles per core
WCOL = 5120          # columns of znT each core needs
TEMP_INV = 10.0
QS = 16.0            # host quantization scale: q = fp8(QS * zn)
SC = TEMP_INV / (QS * QS)        # ACT exp scale: exp(SC * sim')
LN2 = float(np.log(2.0))
S1D = SC * 128.0 / LN2           # DVE bf16-schraudolph slope
S2D = 16248.64                   # bias (incl. mean-error calibration)
S1D8 = SC * 4.0 / LN2            # fp8e5-schraudolph slope (c1 window)
S2D8 = 59.7787
MASKV = -1536.0                  # band mask in PSUM units: SC*MASKV = -60
MASKD = -40000.0                 # diag mask added to master (x' < 0 -> 0)
W0 = 1024            # DVE width in c0 (covers all diag subtiles)
W2 = 512             # DVE width in c1
WD = W0 + W2

F32 = mybir.dt.float32
BF16 = mybir.dt.bfloat16
FP8E4 = mybir.dt.float8e4
FP8E5 = mybir.dt.float8e5
I16 = mybir.dt.int16
I8 = mybir.dt.int8
AF = mybir.ActivationFunctionType
ALU = mybir.AluOpType
DR = mybir.MatmulPerfMode.DoubleRow

# colsum pieces: (cc, src, off, t0, nt) -> cs_sb[:, 512*p : 512*(p+1)]
# covers local col chunk 1024 + 512*cc over row tiles [t0, t0+nt);
# host adds the pieces of each chunk.
#   src: 0=E0a (fp8), 1=Ed bf16 (c1 window, off absolute), 2=E1a, 3=Eb
_CHUNKS = [
    (0, 0, 0), (1, 0, 512), (2, 4, 0), (3, 2, 0),
    (4, 2, 512), (5, 2, 1024), (6, 3, 0),
]
PIECES = (
    [(cc, src, off, 0, 4) for cc, src, off in _CHUNKS]     # A: tiles 0-3
    + [(cc, src, off, 4, 4) for cc, src, off in _CHUNKS]   # B: tiles 4-7
    + [(7, 3, 512, 4, 4)]
)
NP_ = len(PIECES)

_nc_cache = None


def _build():
    nc = bacc.Bacc(None, target_bir_lowering=False)
    zt = nc.dram_tensor("zt", [P, 2, WCOL], FP8E4, kind="ExternalInput")
    mtri_in = nc.dram_tensor("mtri", [P, P], F32, kind="ExternalInput")
    maskd_in = nc.dram_tensor("maskd", [P, P], F32, kind="ExternalInput")
    out_acc = nc.dram_tensor("out_acc", [P, IT * 6], F32, kind="ExternalOutput")
    out_cs = nc.dram_tensor("out_cs", [1, NP_ * 512], BF16, kind="ExternalOutput")

    with (
        tile.TileContext(nc) as tc,
        tc.tile_pool(name="sb", bufs=1) as sb,
        tc.tile_pool(name="big", bufs=3, space="PSUM") as bigp,
        tc.tile_pool(name="csp", bufs=1, space="PSUM") as cspp,
    ):
        zts = sb.tile([P, 2, WCOL], FP8E4)
        Ed = sb.tile([P, IT, W0], BF16)        # c0 DVE E (schraudolph bits)
        E1d = sb.tile([P, IT, W2], FP8E5)      # c1 DVE E (fp8e5 bits)
        E0a = sb.tile([P, IT, 2048 - W0], FP8E5)
        E1a = sb.tile([P, IT, 2048 - W2], FP8E5)
        Eb = sb.tile([P, IT, 1024], FP8E5)
        acc = sb.tile([P, IT, 6], F32)
        cs_sb = sb.tile([P, NP_ * 512], BF16)
        master = sb.tile([P, 2048], F32)
        mtri = sb.tile([P, P], F32)
        maskd = sb.tile([P, P], F32)
        ones2 = sb.tile([P, 2, P], FP8E5)      # DoubleRow colsum lhsT
        ones1b = sb.tile([P, P], BF16)         # plain bf16 colsum lhsT

        # memsets first (no deps; master must beat the first STT)
        nc.vector.memset(master, float(S2D))
        nc.vector.memset(acc, 0.0)
        nc.vector.memset(ones2, 1.0)
        nc.vector.memset(ones1b, 1.0)

        # input DMAs on the two fast HWDGE queues, ordered by first use
        # (SWDGE/gpsimd descriptor gen is too slow for the early slabs)
        nc.sync.dma_start(out=zts[:, :, 0:512], in_=zt[:, :, 0:512])
        nc.sync.dma_start(out=zts[:, :, 512:1024], in_=zt[:, :, 512:1024])
        nc.scalar.dma_start(out=zts[:, :, 1024:2048], in_=zt[:, :, 1024:2048])
        nc.scalar.dma_start(out=zts[:, :, 2048:3072], in_=zt[:, :, 2048:3072])
        nc.sync.dma_start(out=zts[:, :, 3072:4096], in_=zt[:, :, 3072:4096])
        nc.scalar.dma_start(out=zts[:, :, 4096:WCOL], in_=zt[:, :, 4096:WCOL])
        nc.sync.dma_start(out=maskd, in_=maskd_in[:, :])
        nc.sync.dma_start(out=mtri, in_=mtri_in[:, :])
        nc.gpsimd.memset(Eb, 0.0)   # band tail stays zero for full-width cs

        # master: S2D everywhere; diag-mask block at cols [1024, 1152)
        # (row tile i's STT window [1024-128i, ...) puts the block exactly
        # over its self-sim diagonal subtile)
        nc.vector.tensor_add(master[:, 1024:1152], master[:, 1024:1152], maskd)

        cps1 = cspp.tile([P, 512], F32, name="cps1")
        cps2 = cspp.tile([P, 512], F32, name="cps2")
        cps = [cps1, cps2]

        def lhsT(i):
            return zts[:, :, i * P : (i + 1) * P]

        def emit_main(i):
            # ---- c0: cols [0, 2048) ----
            a0 = bigp.tile([P, W0], F32, tag="ps", name=f"a0_{i}")
            a1 = bigp.tile([P, 2048 - W0], F32, tag="ps", name=f"a1_{i}")
            for n in range(0, W0, 512):
                nc.tensor.matmul(
                    a0[:, n : n + 512], lhsT=lhsT(i),
                    rhs=zts[:, :, n : n + 512],
                    start=True, stop=True, perf_mode=DR,
                )
            for n in range(W0, 2048, 512):
                nc.tensor.matmul(
                    a1[:, n - W0 : n - W0 + 512], lhsT=lhsT(i),
                    rhs=zts[:, :, n : n + 512],
                    start=True, stop=True, perf_mode=DR,
                )
            # DVE: bf16 schraudolph, diag mask fused via master window
            nc.vector.scalar_tensor_tensor(
                out=Ed[:, i, 0:W0].bitcast(I16), in0=a0[:, :],
                scalar=float(S1D),
                in1=master[:, 1024 - P * i : 1024 - P * i + W0],
                op0=ALU.mult, op1=ALU.add,
            )
            nc.scalar.activation(
                out=E0a[:, i, :], in_=a1[:, :], func=AF.Exp, scale=float(SC),
                accum_out=acc[:, i, 1:2],
            )

            # ---- c1: cols [2048, 4096) ----
            b0 = bigp.tile([P, 1024], F32, tag="ps", name=f"b0_{i}")
            b1 = bigp.tile([P, 1024], F32, tag="ps", name=f"b1_{i}")
            for n in range(0, 1024, 512):
                nc.tensor.matmul(
                    b0[:, n : n + 512], lhsT=lhsT(i),
                    rhs=zts[:, :, 2048 + n : 2048 + n + 512],
                    start=True, stop=True, perf_mode=DR,
                )
            for n in range(0, 1024, 512):
                nc.tensor.matmul(
                    b1[:, n : n + 512], lhsT=lhsT(i),
                    rhs=zts[:, :, 3072 + n : 3072 + n + 512],
                    start=True, stop=True, perf_mode=DR,
                )
            nc.vector.tensor_scalar(
                out=E1d[:, i, :].bitcast(I8), in0=b0[:, 0:W2],
                scalar1=float(S1D8), scalar2=float(S2D8),
                op0=ALU.mult, op1=ALU.add,
            )
            nc.vector.reduce_sum(
                acc[:, i, 0:1], Ed[:, i, :], axis=mybir.AxisListType.X
            )
            nc.vector.reduce_sum(
                acc[:, i, 2:3], E1d[:, i, :], axis=mybir.AxisListType.X
            )
            nc.scalar.activation(
                out=E1a[:, i, 0 : 1024 - W2], in_=b0[:, W2:1024],
                func=AF.Exp, scale=float(SC), accum_out=acc[:, i, 3:4],
            )
            nc.scalar.activation(
                out=E1a[:, i, 1024 - W2 : 2048 - W2], in_=b1[:, :],
                func=AF.Exp, scale=float(SC), accum_out=acc[:, i, 4:5],
            )

            # ---- band: cols [4096, 4096 + 128(i+1)) ----
            # boundary subtile merged into the last bank-contained MM
            wb = P * (i + 1)
            cb = bigp.tile([P, 1024], F32, tag="ps", name=f"cb_{i}")
            full = wb - P
            n = 0
            while n < wb:
                w = min(512 - (n % 512), wb - n)
                nc.tensor.matmul(
                    cb[:, n : n + w], lhsT=lhsT(i),
                    rhs=zts[:, :, 4096 + n : 4096 + n + w],
                    start=True, stop=True, perf_mode=DR,
                )
                n += w
            nc.vector.tensor_add(
                cb[:, full : full + P], cb[:, full : full + P], mtri
            )
            nc.scalar.activation(
                out=Eb[:, i, 0:wb], in_=cb[:, 0:wb], func=AF.Exp,
                scale=float(SC), accum_out=acc[:, i, 5:6],
            )

        def emit_piece(p, bank, engine):
            cc, src, off, t0, nt = PIECES[p]
            dst = cps[bank]
            if src == 1:
                # bf16 plain chain over the piece's row tiles
                for k in range(nt):
                    nc.tensor.matmul(
                        dst[:, :], lhsT=ones1b,
                        rhs=Ed[:, t0 + k, off : off + 512],
                        start=(k == 0), stop=(k == nt - 1),
                    )
            else:
                S = {0: E0a, 2: E1a, 3: Eb, 4: E1d}[src]
                for k in range(nt // 2):
                    nc.tensor.matmul(
                        dst[:, :], lhsT=ones2,
                        rhs=S[:, t0 + 2 * k : t0 + 2 * k + 2, off : off + 512],
                        start=(k == 0), stop=(k == nt // 2 - 1), perf_mode=DR,
                    )
            seg = cs_sb[:, 512 * p : 512 * (p + 1)]
            if engine == "v":
                nc.vector.tensor_copy(out=seg, in_=dst[:, :])
            else:
                nc.scalar.copy(seg, dst[:, :])

        # ---- main schedule: A pieces (tiles 0-3) ride iters 4-7 ----
        SCHED = {4: [0, 1], 5: [2, 3], 6: [4, 5, 6], 7: []}
        for i in range(IT):
            emit_main(i)
            for j, p in enumerate(SCHED.get(i, [])):
                emit_piece(p, j % 2, "v")
        # tail: B pieces (tiles 4-7), copies split across both exp engines
        for j, p in enumerate([14] + list(range(7, 14))):
            emit_piece(p, j % 2, "v" if j % 2 else "s")

        nc.scalar.dma_start(
            out=out_acc[:, :], in_=acc.rearrange("p i s -> p (i s)")
        )
        nc.sync.dma_start(out=out_cs[0:1, :], in_=cs_sb[0:1, :])

    nc.finalize()
    return nc


def _get_nc():
    global _nc_cache
    if _nc_cache is None:
        _nc_cache = _build()
    return _nc_cache


def _prep_inputs(z: np.ndarray):
    """z: [M, D] float32 (unnormalized). Returns per-core input maps and
    host-side positive-pair cosines."""
    nrm = np.sqrt((z.astype(np.float64) ** 2).sum(axis=1))
    zn = z / np.maximum(nrm, 1e-8).astype(np.float32)[:, None]
    zn64 = zn.astype(np.float64)
    pos = (zn64 * np.roll(zn64, -N2, axis=0)).sum(axis=1)
    q = (QS * zn).astype(ml_dtypes.float8_e4m3)
    # [128, 2, 8192]: [p, h, col] = q[col, 128h + p]
    big = np.ascontiguousarray(q.T.reshape(2, P, M).transpose(1, 0, 2))
    mtri = np.where(
        np.arange(P)[None, :] >= np.arange(P)[:, None], MASKV, 0.0
    ).astype(np.float32)
    maskd = (MASKD * np.eye(P)).astype(np.float32)
    in_maps = []
    for c in range(NCORES):
        zr = np.roll(big, -1024 * c, axis=2)[:, :, :WCOL]
        in_maps.append(
            {"zt": np.ascontiguousarray(zr), "mtri": mtri, "maskd": maskd}
        )
    return in_maps, pos


def _run_cores(z: np.ndarray, trace: bool = False):
    from concourse.bass_utils import run_bass_kernel_spmd

    nc = _get_nc()
    in_maps, _ = _prep_inputs(np.asarray(z, np.float32))
    return run_bass_kernel_spmd(
        nc, in_maps, core_ids=list(range(NCORES)), trace=trace
    )


def _combine(results, pos):
    total = np.zeros(M, np.float64)
    idx = np.arange(512)
    for c, r in enumerate(results):
        accv = np.asarray(r["out_acc"]).astype(np.float64).reshape(P, IT, 6)
        cs = np.asarray(r["out_cs"]).astype(np.float64)[0]
        base = 1024 * c
        rows = accv.sum(axis=2)  # [P, IT]
        for i in range(IT):
            g = (base + i * P + np.arange(P)) % M
            total[g] += rows[:, i]
        for p, (cc, src, off, t0, nt) in enumerate(PIECES):
            j = 1024 + 512 * cc + idx
            total[(base + j) % M] += cs[512 * p : 512 * (p + 1)]

    total += np.exp(TEMP_INV * pos)
    lse = np.log(total)
    return np.float32((lse - TEMP_INV * pos).mean())


def kernel(z1: np.ndarray, z2: np.ndarray) -> np.ndarray:
    from concourse.bass_utils import run_bass_kernel_spmd

    z = np.concatenate(
        [np.asarray(z1, np.float32), np.asarray(z2, np.float32)], axis=0
    )
    nc = _get_nc()
    in_maps, pos = _prep_inputs(z)
    res = run_bass_kernel_spmd(nc, in_maps, core_ids=list(range(NCORES)))
    return _combine(res.results, pos)
